# revision 1
# baseline (speedup 1.0000x reference)
"""BiquadWQFilter Trainium2 kernel — 8-core data-parallel (8 voices/core).

Algorithm per core ([128, 8192] layout; partition = contiguous 8192-sample
segment of one voice, 16 segments/voice):
  1. ACT/DVE coefficient pipeline (clip, exp, sin, cos, recip).
  2. Leaf pass: chunked 3-stream 2nd-order scan (particular + 2 homogeneous
     bases) over T0-sample chunks -> per-chunk affine summaries.
  3. Hierarchical stitch of chunk summaries (2x2 affine chain scan, 3 levels)
     + cross-partition chain via small DRAM-bridged shuffle.
  4. Downsweep -> per-chunk initial states; pass 2 recomputes y in place.
  5. Fused FIR tail: y_ab = -0.5*nb2*(y + 2*y[n-1] + y[n-2]).
"""
import sys

if '/opt/trn_rl_repo' not in sys.path:
    sys.path.insert(0, '/opt/trn_rl_repo')

import numpy as np
import concourse.bass as bass
import concourse.mybir as mybir

F32 = mybir.dt.float32
AF = mybir.ActivationFunctionType
OP = mybir.AluOpType

SR = 44100
MIN_W = 2.0 * np.pi * 20.0 / SR
MAX_W = 2.0 * np.pi * 8000.0 / SR
MIN_Q = 0.7071
MAX_Q = 8.0
EPS = 0.001
LW0, LW1 = float(np.log(MIN_W)), float(np.log(MAX_W))
LQ0, LQ1 = float(np.log(MIN_Q)), float(np.log(MAX_Q))

B, NSAMP = 64, 131072
NCORES = 8
P, N = 128, 8192
T0, LEVELS = 32, (8, 8, 4)
N_VOICES = 8


def ts_view(ap, stride, t_l):
    return ap.rearrange("p (w r) -> p w r", r=t_l * stride)


def build(nc, tc, x_d, w_d, q_d, o_d, P=P, N=N, T0=T0, LEVELS=LEVELS,
          n_voices=N_VOICES, cseg=512, uniq=""):
    n_vps = P // n_voices
    W0 = N // T0
    Ks = [W0]
    for T in LEVELS:
        assert Ks[-1] % T == 0
        Ks.append(Ks[-1] // T)
    assert Ks[-1] == 1

    with (
        tc.tile_pool(name="big", bufs=1) as pbig,
        tc.tile_pool(name="coeff", bufs=1) as pc,
        tc.tile_pool(name="roll", bufs=1) as pr,
        tc.tile_pool(name="tmp", bufs=2) as pt,
        tc.tile_pool(name="stitch", bufs=1) as ps,
    ):
        xy = pbig.tile([P, N], F32, tag="xy")
        na1 = pbig.tile([P, N], F32, tag="na1")
        na2 = pbig.tile([P, N], F32, tag="na2")
        nb2 = pbig.tile([P, N], F32, tag="nb2")

        nc.sync.dma_start(xy[:], x_d[:])

        _consts: dict[float, object] = {}

        def constp(val):
            val = float(val)
            if val not in _consts:
                t = ps.tile([P, 1], F32, name=f"c{len(_consts)}", tag=f"c{len(_consts)}")
                nc.vector.memset(t[:], val)
                _consts[val] = t
            return _consts[val][:]

        # ---------------- coefficients ----------------
        for s in range(0, N, cseg):
            sl = slice(s, s + cseg)
            wm = pc.tile([P, cseg], F32, tag="wm")
            qm = pc.tile([P, cseg], F32, tag="qm")
            nc.sync.dma_start(wm[:], w_d[:, sl])
            nc.sync.dma_start(qm[:], q_d[:, sl])
            nc.vector.tensor_scalar(wm[:], wm[:], EPS, 1.0 - EPS, OP.max, OP.min)
            nc.vector.tensor_scalar(qm[:], qm[:], EPS, 1.0 - EPS, OP.max, OP.min)
            wv = pc.tile([P, cseg], F32, tag="wv")
            nc.scalar.activation(wv[:], wm[:], AF.Exp, bias=constp(LW0),
                                 scale=constp(LW1 - LW0))
            iq = pc.tile([P, cseg], F32, tag="iq")
            nc.scalar.activation(iq[:], qm[:], AF.Exp,
                                 bias=constp(float(np.log(0.5)) - LQ0),
                                 scale=constp(-(LQ1 - LQ0)))
            sn = pc.tile([P, cseg], F32, tag="sn")
            nc.scalar.activation(sn[:], wv[:], AF.Sin)
            cs = pc.tile([P, cseg], F32, tag="cs")
            nc.scalar.activation(cs[:], wv[:], AF.Sin, bias=constp(float(np.pi / 2)))
            al = pc.tile([P, cseg], F32, tag="al")
            nc.vector.tensor_tensor(al[:], sn[:], iq[:], OP.mult)
            a0i = pc.tile([P, cseg], F32, tag="a0i")
            nc.vector.tensor_scalar_add(a0i[:], al[:], 1.0)
            nc.vector.reciprocal(a0i[:], a0i[:])
            nc.vector.scalar_tensor_tensor(na1[:, sl], cs[:], 2.0, a0i[:], OP.mult, OP.mult)
            nc.vector.scalar_tensor_tensor(na2[:, sl], al[:], -1.0, a0i[:], OP.add, OP.mult)
            nc.vector.scalar_tensor_tensor(nb2[:, sl], cs[:], -1.0, a0i[:], OP.add, OP.mult)

        # ---------------- pass 1: leaf 3-stream ----------------
        na1v = ts_view(na1[:], 1, T0)
        na2v = ts_view(na2[:], 1, T0)
        xyv = ts_view(xy[:], 1, T0)

        def roll_tile(tag, val):
            t = pr.tile([P, W0], F32, name=tag, tag=tag)
            nc.vector.memset(t[:], val)
            return t

        y1, y2 = roll_tile("y1", 0.0), roll_tile("y2", 0.0)
        k1, k2 = roll_tile("k1", 1.0), roll_tile("k2", 0.0)
        m1, m2 = roll_tile("m1", 0.0), roll_tile("m2", 1.0)

        for t in range(T0):
            a1 = na1v[:, :, t]
            a2 = na2v[:, :, t]
            xt = xyv[:, :, t]
            t1 = pt.tile([P, W0], F32, tag="t1")
            t2 = pt.tile([P, W0], F32, tag="t2")
            nc.vector.tensor_tensor(t1[:], a1, y1[:], OP.mult)
            nc.vector.tensor_tensor(t2[:], a2, y2[:], OP.mult)
            nc.vector.tensor_tensor(t1[:], t1[:], t2[:], OP.add)
            nc.vector.tensor_tensor(y2[:], t1[:], xt, OP.add)
            y1, y2 = y2, y1
            t3 = pt.tile([P, W0], F32, tag="t3")
            t4 = pt.tile([P, W0], F32, tag="t4")
            nc.vector.tensor_tensor(t3[:], a1, k1[:], OP.mult)
            nc.vector.tensor_tensor(t4[:], a2, k2[:], OP.mult)
            nc.vector.tensor_tensor(k2[:], t3[:], t4[:], OP.add)
            k1, k2 = k2, k1
            t5 = pt.tile([P, W0], F32, tag="t5")
            t6 = pt.tile([P, W0], F32, tag="t6")
            nc.vector.tensor_tensor(t5[:], a1, m1[:], OP.mult)
            nc.vector.tensor_tensor(t6[:], a2, m2[:], OP.mult)
            nc.vector.tensor_tensor(m2[:], t5[:], t6[:], OP.add)
            m1, m2 = m2, m1

        chain_in = [k1[:], m1[:], k2[:], m2[:], y1[:], y2[:]]
        stride, off = 1, 0

        # ---------------- stitch upsweep ----------------
        streams_per_level = []
        for li, T in enumerate(LEVELS):
            K = Ks[li]
            Wl = K // T
            outs = [ps.tile([P, K], F32, name=f"L{li}s{i}", tag=f"L{li}s{i}")
                    for i in range(6)]
            S01, S02, U1, U2, V1, V2 = outs
            ovs = [ts_view(o[:], 1, T) for o in outs]
            for g in range(T):
                Ag = [ts_view(a, stride, T)[:, :, off + g * stride] for a in chain_in]
                A, Bm, C, D, e, f = Ag
                if g == 0:
                    nc.vector.tensor_copy(ovs[0][:, :, 0], e)
                    nc.vector.tensor_copy(ovs[1][:, :, 0], f)
                    nc.vector.tensor_copy(ovs[2][:, :, 0], A)
                    nc.vector.tensor_copy(ovs[3][:, :, 0], C)
                    nc.vector.tensor_copy(ovs[4][:, :, 0], Bm)
                    nc.vector.tensor_copy(ovs[5][:, :, 0], D)
                    continue
                ps1, ps2, pu1, pu2, pv1, pv2 = [ov[:, :, g - 1] for ov in ovs]
                t1 = pt.tile([P, Wl], F32, tag="st1")
                t2 = pt.tile([P, Wl], F32, tag="st2")
                nc.vector.tensor_tensor(t1[:], A, ps1, OP.mult)
                nc.vector.tensor_tensor(t2[:], Bm, ps2, OP.mult)
                nc.vector.tensor_tensor(t1[:], t1[:], t2[:], OP.add)
                nc.vector.tensor_tensor(ovs[0][:, :, g], t1[:], e, OP.add)
                nc.vector.tensor_tensor(t1[:], C, ps1, OP.mult)
                nc.vector.tensor_tensor(t2[:], D, ps2, OP.mult)
                nc.vector.tensor_tensor(t1[:], t1[:], t2[:], OP.add)
                nc.vector.tensor_tensor(ovs[1][:, :, g], t1[:], f, OP.add)
                nc.vector.tensor_tensor(t1[:], A, pu1, OP.mult)
                nc.vector.tensor_tensor(t2[:], Bm, pu2, OP.mult)
                nc.vector.tensor_tensor(ovs[2][:, :, g], t1[:], t2[:], OP.add)
                nc.vector.tensor_tensor(t1[:], C, pu1, OP.mult)
                nc.vector.tensor_tensor(t2[:], D, pu2, OP.mult)
                nc.vector.tensor_tensor(ovs[3][:, :, g], t1[:], t2[:], OP.add)
                nc.vector.tensor_tensor(t1[:], A, pv1, OP.mult)
                nc.vector.tensor_tensor(t2[:], Bm, pv2, OP.mult)
                nc.vector.tensor_tensor(ovs[4][:, :, g], t1[:], t2[:], OP.add)
                nc.vector.tensor_tensor(t1[:], C, pv1, OP.mult)
                nc.vector.tensor_tensor(t2[:], D, pv2, OP.mult)
                nc.vector.tensor_tensor(ovs[5][:, :, g], t1[:], t2[:], OP.add)
            streams_per_level.append(outs)
            chain_in = [o[:] for o in (U1, V1, U2, V2, S01, S02)]
            stride, off = T, T - 1

        # ---------------- cross-partition chain via DRAM bridge ----------------
        packed = ps.tile([P, 8], F32, tag="packed")
        for i, a in enumerate(chain_in):
            nc.vector.tensor_copy(packed[:, i:i + 1], a[:, off:off + 1])
        pk_d = nc.dram_tensor(f"l4_packed_scratch{uniq}", [P, 6], F32,
                              kind="Internal").ap()
        nc.sync.dma_start(pk_d[:], packed[:, 0:6])
        lin = ps.tile([n_voices, n_vps * 6], F32, tag="lin")
        nc.sync.dma_start(lin[:], pk_d.rearrange("(v s) i -> v (s i)", s=n_vps))
        pre_p = ps.tile([n_voices, n_vps * 2], F32, tag="prep")
        nc.vector.memset(pre_p[:, 0:2], 0.0)
        for s in range(1, n_vps):
            b = (s - 1) * 6
            u1c, v1c, u2c, v2c, s1c, s2c = [lin[:, b + i:b + i + 1] for i in range(6)]
            p1c = pre_p[:, 2 * (s - 1):2 * (s - 1) + 1]
            p2c = pre_p[:, 2 * (s - 1) + 1:2 * s]
            t1 = pt.tile([n_voices, 1], F32, tag="l4a")
            t2 = pt.tile([n_voices, 1], F32, tag="l4b")
            nc.vector.tensor_tensor(t1[:], u1c, p1c, OP.mult)
            nc.vector.tensor_tensor(t2[:], v1c, p2c, OP.mult)
            nc.vector.tensor_tensor(t1[:], t1[:], t2[:], OP.add)
            nc.vector.tensor_tensor(pre_p[:, 2 * s:2 * s + 1], t1[:], s1c, OP.add)
            nc.vector.tensor_tensor(t1[:], u2c, p1c, OP.mult)
            nc.vector.tensor_tensor(t2[:], v2c, p2c, OP.mult)
            nc.vector.tensor_tensor(t1[:], t1[:], t2[:], OP.add)
            nc.vector.tensor_tensor(pre_p[:, 2 * s + 1:2 * s + 2], t1[:], s2c, OP.add)
        pp_d = nc.dram_tensor(f"l4_pre_scratch{uniq}", [n_voices, n_vps * 2], F32,
                              kind="Internal").ap()
        nc.sync.dma_start(pp_d[:], pre_p[:])
        Pp = ps.tile([P, 2], F32, tag="Pp")
        nc.sync.dma_start(Pp[:], pp_d.rearrange("v (s c) -> (v s) c", c=2))
        Pp1 = Pp[:, 0:1]
        Pp2 = Pp[:, 1:2]

        # ---------------- downsweep ----------------
        pre1, pre2 = Pp1, Pp2
        for li in range(len(LEVELS) - 1, -1, -1):
            T = LEVELS[li]
            K = Ks[li]
            S01, S02, U1, U2, V1, V2 = streams_per_level[li]
            b1 = ps.tile([P, K], F32, name=f"b1_{li}", tag=f"b1_{li}")
            b2 = ps.tile([P, K], F32, name=f"b2_{li}", tag=f"b2_{li}")
            b1v, b2v = ts_view(b1[:], 1, T), ts_view(b2[:], 1, T)
            for g in range(T):
                nc.vector.tensor_copy(b1v[:, :, g], pre1)
                nc.vector.tensor_copy(b2v[:, :, g], pre2)
            t1 = pt.tile([P, K], F32, name=f"dt_{li}", tag=f"dt_{li}")
            nc.vector.tensor_tensor(t1[:], U1[:], b1[:], OP.mult)
            nc.vector.tensor_tensor(S01[:], S01[:], t1[:], OP.add)
            nc.vector.tensor_tensor(t1[:], V1[:], b2[:], OP.mult)
            nc.vector.tensor_tensor(S01[:], S01[:], t1[:], OP.add)
            nc.vector.tensor_tensor(t1[:], U2[:], b1[:], OP.mult)
            nc.vector.tensor_tensor(S02[:], S02[:], t1[:], OP.add)
            nc.vector.tensor_tensor(t1[:], V2[:], b2[:], OP.mult)
            nc.vector.tensor_tensor(S02[:], S02[:], t1[:], OP.add)
            npre1 = ps.tile([P, K], F32, name=f"p1_{li}", tag=f"p1_{li}")
            npre2 = ps.tile([P, K], F32, name=f"p2_{li}", tag=f"p2_{li}")
            if K > 1:
                nc.vector.tensor_copy(npre1[:, 1:K], S01[:, 0:K - 1])
                nc.vector.tensor_copy(npre2[:, 1:K], S02[:, 0:K - 1])
            nv1, nv2 = ts_view(npre1[:], 1, T), ts_view(npre2[:], 1, T)
            nc.vector.tensor_copy(nv1[:, :, 0], pre1)
            nc.vector.tensor_copy(nv2[:, :, 0], pre2)
            pre1, pre2 = npre1[:], npre2[:]

        # ---------------- pass 2 ----------------
        for t in range(T0):
            a1 = na1v[:, :, t]
            a2 = na2v[:, :, t]
            if t == 0:
                ym1, ym2 = pre1, pre2
            elif t == 1:
                ym1, ym2 = xyv[:, :, 0], pre1
            else:
                ym1, ym2 = xyv[:, :, t - 1], xyv[:, :, t - 2]
            t1 = pt.tile([P, W0], F32, tag="q1")
            t2 = pt.tile([P, W0], F32, tag="q2")
            nc.vector.tensor_tensor(t1[:], a1, ym1, OP.mult)
            nc.vector.tensor_tensor(t2[:], a2, ym2, OP.mult)
            nc.vector.tensor_tensor(t1[:], t1[:], t2[:], OP.add)
            nc.vector.tensor_tensor(xyv[:, :, t], xyv[:, :, t], t1[:], OP.add)

        # ---------------- FIR ----------------
        u = pbig.tile([P, N], F32, name="u", tag="na1")
        nc.vector.tensor_tensor(u[:, 2:N], xy[:, 2:N], xy[:, 0:N - 2], OP.add)
        nc.vector.tensor_tensor(u[:, 0:1], xy[:, 0:1], Pp2, OP.add)
        nc.vector.tensor_tensor(u[:, 1:2], xy[:, 1:2], Pp1, OP.add)
        nc.vector.scalar_tensor_tensor(u[:, 1:N], xy[:, 0:N - 1], 2.0, u[:, 1:N],
                                       OP.mult, OP.add)
        nc.vector.scalar_tensor_tensor(u[:, 0:1], Pp1, 2.0, u[:, 0:1],
                                       OP.mult, OP.add)
        nc.vector.scalar_tensor_tensor(u[:], u[:], -0.5, nb2[:], OP.mult, OP.mult)
        nc.sync.dma_start(o_d[:], u[:])


_CACHE = {}


def _get_nc(repeats=1):
    key = ("nc", repeats)
    if key in _CACHE:
        return _CACHE[key]
    import concourse.bacc as bacc
    import concourse.tile as tile
    nc = bacc.Bacc("TRN2", target_bir_lowering=False, debug=False,
                   num_devices=NCORES)
    x_d = nc.dram_tensor("x", [P, N], F32, kind="ExternalInput").ap()
    w_d = nc.dram_tensor("w", [P, N], F32, kind="ExternalInput").ap()
    q_d = nc.dram_tensor("q", [P, N], F32, kind="ExternalInput").ap()
    o_d = nc.dram_tensor("o", [P, N], F32, kind="ExternalOutput").ap()
    with tile.TileContext(nc) as tc:
        if repeats == 1:
            build(nc, tc, x_d, w_d, q_d, o_d)
        else:
            with tc.For_i(0, repeats, 1):
                build(nc, tc, x_d, w_d, q_d, o_d)
    nc.finalize()
    _CACHE[key] = nc
    return nc


def kernel(x, w_mod_sig, q_mod_sig):
    from concourse import bass_utils
    nc = _get_nc()
    in_maps = []
    for i in range(NCORES):
        sl = slice(i * (B // NCORES), (i + 1) * (B // NCORES))
        in_maps.append({
            "x": np.ascontiguousarray(x[sl]).reshape(P, N),
            "w": np.ascontiguousarray(w_mod_sig[sl]).reshape(P, N),
            "q": np.ascontiguousarray(q_mod_sig[sl]).reshape(P, N),
        })
    res = bass_utils.run_bass_kernel_spmd(nc, in_maps, core_ids=list(range(NCORES)))
    out = np.empty((B, NSAMP), np.float32)
    for i in range(NCORES):
        sl = slice(i * (B // NCORES), (i + 1) * (B // NCORES))
        out[sl] = res.results[i]["o"].reshape(B // NCORES, NSAMP)
    return out
